# revision 1
# baseline (speedup 1.0000x reference)
"""BitLinear (ternary weight quantization + linear) on 8 TRN2 NeuronCores.

y = x @ w_eff.T with w_eff = clip(round(w/scale), -1, 1) * scale,
scale = clamp(mean |w| per row, 1e-5).

Sharding: column-parallel — weight rows (out_features) split 8 ways; each
core computes y[:, shard] for the full x; host concatenates. Quantization
is per-output-row, so it is fully local to a shard.

Matmul runs in fp32r (TF32-like, 11-bit mantissa, full PE rate on TRN2);
measured end-to-end error vs the fp32 reference is ~2e-4 absmax-relative.

Per-core dataflow:
  W phase: for each 128-row chunk of the weight shard, compute the row
  scale (|w| row-sum fused into the Abs activation), build
  w_eff = (w > scale/2)*scale - (w < -scale/2)*scale on the DVE (the
  strict > matches round-half-even semantics of round(w/scale) at the
  0.5 boundary), round to fp32r, PE-transpose, and keep w_eff^T
  resident in SBUF (fp32r, 8 MB).
  X phase: stream 64 row-tiles of x; round to fp32r on the scalar
  engine, PE-transpose into [d_in, row] layout (4 transposes batched
  per PSUM bank, evicted into per-k-group sub-tiles so matmuls start
  early), then 2x16 accumulating N=512 matmuls per tile against the
  resident w_eff^T; evict PSUM via the scalar engine and DMA out.
"""

import numpy as np

import concourse.bass as bass
import concourse.mybir as mybir
import concourse.tile as tile
from concourse import bacc
from concourse.bass_utils import run_bass_kernel_spmd
from concourse.masks import make_identity

F32 = mybir.dt.float32
F32R = mybir.dt.float32r

# Problem shape (hardcoded per contract)
B, S, D_IN, D_OUT = 4, 2048, 2048, 8192
NCORES = 8
R = B * S                 # 8192 rows of x
O = D_OUT // NCORES       # 1024 out features per core
K_SUB = D_IN // 128       # 16 contraction sub-tiles
M_TILES = R // 128        # 64 row tiles
O_TILES = O // 128        # 8 weight row-tiles per core
N_SLICE = 512             # psum bank width (fp32)
N_SLICES = O // N_SLICE   # 2
TGRP = 4                  # transposes batched per psum bank


def _build():
    nc = bacc.Bacc(None, target_bir_lowering=False)

    x_d = nc.dram_tensor("x", [R, D_IN], F32, kind="ExternalInput")
    w_d = nc.dram_tensor("w", [O, D_IN], F32, kind="ExternalInput")
    y_d = nc.dram_tensor("y", [R, O], F32, kind="ExternalOutput")

    with tile.TileContext(nc) as tc:
        with (
            tc.tile_pool(name="const", bufs=1) as const,
            tc.tile_pool(name="wt", bufs=1) as wtp,
            tc.tile_pool(name="ws", bufs=1) as ws,
            tc.tile_pool(name="xs", bufs=3) as xs,
            tc.tile_pool(name="ys", bufs=3) as ysp,
            tc.tile_pool(name="ps", bufs=3, space="PSUM") as ps,
            tc.tile_pool(name="ymm", bufs=4, space="PSUM") as ymm,
        ):
            ident_f = const.tile([128, 128], F32)
            make_identity(nc, ident_f[:])
            ident = const.tile([128, 128], F32R)
            nc.vector.tensor_copy(ident[:], ident_f[:])

            # W^T resident in SBUF, one tile per n-slice:
            # wts[n][:, k, o'] = w_eff^T[i_sub, k, n*512 + o']
            wts = [
                wtp.tile([128, K_SUB, N_SLICE], F32R, name=f"wt{n}")
                for n in range(N_SLICES)
            ]

            def w_chunk(a):
                """Quantize + transpose weight rows a*128..(a+1)*128."""
                w_in = ws.tile([128, D_IN], F32, tag="w_in", bufs=2,
                               name=f"w_in_{a}")
                nc.sync.dma_start(w_in[:], w_d[a * 128 : (a + 1) * 128, :])

                # |w| row-sum fused into the Abs activation; the abs
                # values land in the buffer later reused for `neg`
                absdump = ws.tile([128, D_IN], F32, tag="w_neg",
                                  name=f"absdump_{a}")
                ssum = ws.tile([128, 1], F32, tag="w_sum", name=f"ssum_{a}")
                nc.scalar.activation(
                    absdump[:], w_in[:],
                    mybir.ActivationFunctionType.Abs,
                    accum_out=ssum[:],
                )
                scale = ws.tile([128, 1], F32, tag="w_scale",
                                name=f"scale_{a}")
                nc.vector.tensor_scalar(
                    out=scale[:], in0=ssum[:], scalar1=1.0 / D_IN,
                    scalar2=1e-5, op0=mybir.AluOpType.mult,
                    op1=mybir.AluOpType.max,
                )
                hpos = ws.tile([128, 1], F32, tag="w_hpos", name=f"hp_{a}")
                hneg = ws.tile([128, 1], F32, tag="w_hneg", name=f"hn_{a}")
                nc.vector.tensor_scalar_mul(hpos[:], scale[:], 0.5)
                nc.vector.tensor_scalar_mul(hneg[:], scale[:], -0.5)

                # (w > 0.5*scale)*scale - (w < -0.5*scale)*scale
                pos = ws.tile([128, D_IN], F32, tag="w_pos", name=f"pos_{a}")
                nc.vector.tensor_scalar(
                    out=pos[:], in0=w_in[:], scalar1=hpos[:], scalar2=scale[:],
                    op0=mybir.AluOpType.is_gt, op1=mybir.AluOpType.mult,
                )
                neg = ws.tile([128, D_IN], F32, tag="w_neg", name=f"neg_{a}")
                nc.vector.tensor_scalar(
                    out=neg[:], in0=w_in[:], scalar1=hneg[:], scalar2=scale[:],
                    op0=mybir.AluOpType.is_lt, op1=mybir.AluOpType.mult,
                )
                weff = ws.tile([128, D_IN], F32R, tag="w_eff",
                               name=f"weff_{a}")
                nc.vector.tensor_sub(weff[:], pos[:], neg[:])

                n_idx, o_off = divmod(a * 128, N_SLICE)
                for kg in range(K_SUB // TGRP):
                    pt = ps.tile([128, TGRP * 128], F32, tag="wtps", bufs=2,
                                 name=f"wpt_{a}_{kg}")
                    for j in range(TGRP):
                        k = kg * TGRP + j
                        nc.tensor.transpose(
                            pt[:, j * 128 : (j + 1) * 128].bitcast(F32R),
                            weff[:, k * 128 : (k + 1) * 128],
                            ident[:],
                        )
                    half = TGRP // 2
                    dst = wts[n_idx][:, kg * TGRP : (kg + 1) * TGRP,
                                     o_off : o_off + 128]
                    src = pt[:].rearrange("p (g c) -> p g c", g=TGRP)
                    # both halves on ACT: the W-phase cycle is DVE-paced
                    # (pos+neg+sub), ACT has slack there
                    nc.scalar.copy(dst[:, :half], src[:, :half])
                    nc.scalar.copy(dst[:, half:], src[:, half:])

            def x_stage(m):
                """Load/round/transpose x row-tile m; returns x_t sub-tiles."""
                x_in = xs.tile([128, D_IN], F32, tag="x_in", bufs=2,
                               name=f"x_in_{m}")
                nc.sync.dma_start(x_in[:], x_d[m * 128 : (m + 1) * 128, :])
                x_r = xs.tile([128, D_IN], F32R, tag="x_r", bufs=2,
                              name=f"x_r_{m}")
                nc.scalar.copy(x_r[:], x_in[:])

                x_ts = []
                for kg in range(K_SUB // TGRP):
                    pt = ps.tile([128, TGRP * 128], F32, tag="xtps", bufs=3,
                                 name=f"xpt_{m}_{kg}")
                    for j in range(TGRP):
                        k = kg * TGRP + j
                        nc.tensor.transpose(
                            pt[:, j * 128 : (j + 1) * 128].bitcast(F32R),
                            x_r[:, k * 128 : (k + 1) * 128],
                            ident[:],
                        )
                    x_t = xs.tile(
                        [128, TGRP, 128], F32R, tag=f"x_t{kg}", bufs=5,
                        name=f"x_t{kg}_{m}",
                    )
                    nc.vector.tensor_copy(x_t[:], pt[:])
                    x_ts.append(x_t)
                return x_ts

            def mm_group(m, n, x_ts):
                """One accumulation group + eviction + half-row store."""
                acc = ymm.tile([128, N_SLICE], F32, tag="y_ps",
                               name=f"acc{n}_{m}", bufs=3)
                for k in range(K_SUB):
                    nc.tensor.matmul(
                        acc[:],
                        x_ts[k // TGRP][:, k % TGRP, :],
                        wts[n][:, k, :],
                        start=(k == 0),
                        stop=(k == K_SUB - 1),
                    )
                y_sb = ysp.tile([128, N_SLICE], F32, tag=f"y_sb{n}",
                                name=f"y_sb{n}_{m}", bufs=3)
                nc.scalar.copy(y_sb[:], acc[:])
                nc.sync.dma_start(
                    y_d[m * 128 : (m + 1) * 128,
                        n * N_SLICE : (n + 1) * N_SLICE],
                    y_sb[:],
                )

            # Emission schedule: W chunks 0-3 produce wts[0]; then x tiles
            # 0-3 run their n=0 groups interleaved with W chunks 4-7 (which
            # produce wts[1]) so the PE never starves during W quant; then
            # the n=1 groups catch up; then steady state.
            NPRE = O_TILES - N_SLICE // 128  # 4
            pre_xts = []
            for a in range(4):
                w_chunk(a)
                pre_xts.append(x_stage(a))
            for m in range(NPRE):
                mm_group(m, 0, pre_xts[m])
                if 4 + m < O_TILES:
                    w_chunk(4 + m)
            for m in range(NPRE):
                mm_group(m, 1, pre_xts[m])
            # Steady state, software-pipelined: the next tile's transposes
            # are emitted between the current tile's two matmul groups so
            # the x_t evictions land before the PE needs them.
            prev_m, prev_xts = NPRE, x_stage(NPRE)
            for m in range(NPRE + 1, M_TILES):
                mm_group(prev_m, 0, prev_xts)
                cur_xts = x_stage(m)
                mm_group(prev_m, 1, prev_xts)
                prev_m, prev_xts = m, cur_xts
            mm_group(prev_m, 0, prev_xts)
            mm_group(prev_m, 1, prev_xts)

    nc.compile()
    return nc


_NC_CACHE = None


def _get_nc():
    global _NC_CACHE
    if _NC_CACHE is None:
        _NC_CACHE = _build()
    return _NC_CACHE


def kernel(x: np.ndarray, weight: np.ndarray, _trace: bool = False):
    assert x.shape == (B, S, D_IN) and weight.shape == (D_OUT, D_IN)
    x_flat = np.ascontiguousarray(x.reshape(R, D_IN), dtype=np.float32)
    in_maps = [
        {
            "x": x_flat,
            "w": np.ascontiguousarray(
                weight[c * O : (c + 1) * O], dtype=np.float32
            ),
        }
        for c in range(NCORES)
    ]
    nc = _get_nc()
    res = run_bass_kernel_spmd(
        nc, in_maps, core_ids=list(range(NCORES)), trace=_trace
    )
    y = np.concatenate([res.results[c]["y"] for c in range(NCORES)], axis=1)
    out = y.reshape(B, S, D_OUT)
    if _trace:
        return out, res
    return out



# revision 2
# speedup vs baseline: 1.0245x; 1.0245x over previous
"""BitLinear (ternary weight quantization + linear) on 8 TRN2 NeuronCores.

y = x @ w_eff.T with w_eff = clip(round(w/scale), -1, 1) * scale,
scale = clamp(mean |w| per row, 1e-5).

Key idea vs the fp32r baseline: the quantized weight is ternary, so the
matmul is y[m,o] = scale_o * sum_k q[o,k] * x[m,k] with q in {-1,0,1} --
exactly representable in fp8e4. The PE's fp8 DoubleRow perf mode computes
d = w0*m0 + w1*m1 per cell at ~2x the fp32r/bf16 rate. We spend the pair
slots on precision: pair = (x_hi, x_lo) with x_hi = e4m3(x),
x_lo = e4m3(x - x_hi), against duplicated ternary weights (q, q):
  acc = sum_k q_k * (x_hi_k + x_lo_k) ~= sum_k q_k * x_k
with products exact (e10m10) and fp32 PSUM accumulation. Residual error
~1e-3 absmax-relative, well under the 2e-2 gate, at half the PE time.

Sharding: 2 row-groups x 4 out-groups. Each core handles x rows
r*4096..(r+1)*4096 against w rows c*2048..(c+1)*2048 (out columns), so
per-core DMA is 32 MiB x + 16 MiB w + 16 MiB y(fp16) = 64 MiB, under the
PE time. Quantization is per-output-row -> local to a shard. The per-row
scale is applied at PSUM eviction: acc * so_full (so broadcast across
partitions once via a tiny f32r ones-matmul).

Per-core dataflow:
  W phase (16 o-tiles): scale = clamp(rowsum|w|/2048, 1e-5) via ACT
  Abs+accum; ternary q = (w > s/2) - (w < -s/2) on DVE straight into fp8;
  PE-transpose q (fp8) and evict duplicated into the DoubleRow pair
  layout w_dr[n][ki, ksub, 2, o'] resident in SBUF (8 MiB).
  X phase (32 m-tiles): DMA x fp32, ACT-convert to bf16, PE-transpose
  (bf16), DVE-split hi/lo from PSUM into x_t[ki, ksub, 2, m], then 4
  accumulation groups of 16 DoubleRow matmuls each; evict with the
  per-out-column scale on DVE straight to fp16 and DMA out.
  Schedule: W chunks 0-3 -> n-slice 0 ready; phase A runs m=0..7 at n=0
  while W chunks 4-15 fill the remaining slices; phase B catches up the
  (m<8, n>=1) backlog one group per m-tile.
"""

import numpy as np

import concourse.bass as bass
import concourse.mybir as mybir
import concourse.tile as tile
from concourse import bacc
from concourse.bass_utils import run_bass_kernel_spmd
from concourse.masks import make_identity

F32 = mybir.dt.float32
F32R = mybir.dt.float32r
BF16 = mybir.dt.bfloat16
F16 = mybir.dt.float16
F8 = mybir.dt.float8e4
DR = mybir.MatmulPerfMode.DoubleRow

# Problem shape (hardcoded per contract)
B, S, D_IN, D_OUT = 4, 2048, 2048, 8192
NCORES = 8
RGRP, CGRP = 2, 4          # core grid: row-groups x out-groups
R = B * S                  # 8192 rows of x
R_SH = R // RGRP           # 4096 rows per core
O_SH = D_OUT // CGRP       # 2048 out features per core
K_SUB = D_IN // 128        # 16 contraction sub-tiles
M_TILES = R_SH // 128      # 32 row tiles
O_TILES = O_SH // 128      # 16 weight row-tiles per core
N_SLICE = 512              # psum bank width (fp32)
N_SLICES = O_SH // N_SLICE # 4
NPRE = 8                   # m-tiles that run n=0 only while W fills


def _build():
    nc = bacc.Bacc(None, target_bir_lowering=False)

    x_d = nc.dram_tensor("x", [R_SH, D_IN], F32, kind="ExternalInput")
    w_d = nc.dram_tensor("w", [O_SH, D_IN], F32, kind="ExternalInput")
    y_d = nc.dram_tensor("y", [R_SH, O_SH], F16, kind="ExternalOutput")

    with tile.TileContext(nc) as tc:
        with (
            tc.tile_pool(name="const", bufs=1) as const,
            tc.tile_pool(name="wt", bufs=1) as wtp,
            tc.tile_pool(name="ws", bufs=1) as ws,
            tc.tile_pool(name="xs", bufs=1) as xs,
            tc.tile_pool(name="ys", bufs=1) as ysp,
            tc.tile_pool(name="tp", bufs=1, space="PSUM") as tp,
            tc.tile_pool(name="ac", bufs=1, space="PSUM") as ac,
        ):
            ident_f = const.tile([128, 128], F32)
            make_identity(nc, ident_f[:])
            ident_bf = const.tile([128, 128], BF16)
            nc.vector.tensor_copy(ident_bf[:], ident_f[:])
            ident_fr = const.tile([128, 128], F32R)
            nc.vector.tensor_copy(ident_fr[:], ident_f[:])
            # sel[k, t*128+p] = (k==t): row-selector for the so broadcast
            sel_f = const.tile([4, 512], F32)
            nc.gpsimd.memset(sel_f[:], 0.0)
            nc.gpsimd.affine_select(
                out=sel_f[:].rearrange("p (t j) -> p t j", t=4),
                in_=sel_f[:].rearrange("p (t j) -> p t j", t=4),
                compare_op=mybir.AluOpType.not_equal,
                fill=1.0,
                base=0,
                # expr = p - t; ==0 -> fill 1.0
                pattern=[[-1, 4], [0, 128]],
                channel_multiplier=1,
            )
            sel = const.tile([4, 512], F32R)
            nc.vector.tensor_copy(sel[:], sel_f[:])

            # DoubleRow weight layout, resident: w_dr[n][ki, ksub, pair, o']
            w_dr = [
                wtp.tile([128, K_SUB, 2, N_SLICE], F8, name=f"wdr{n}")
                for n in range(N_SLICES)
            ]
            # so_full[n][p, o'] = scale of out column n*512+o' (any p)
            so_full = [
                wtp.tile([128, N_SLICE], F32, name=f"so{n}")
                for n in range(N_SLICES)
            ]
            so_col = wtp.tile([128, O_TILES], F32R, name="so_col")

            def w_chunk(a):
                """Quantize + transpose weight rows a*128..(a+1)*128."""
                w_in = ws.tile([128, D_IN], F32, tag="w_in", bufs=2,
                               name=f"w_in_{a}")
                nc.sync.dma_start(w_in[:], w_d[a * 128 : (a + 1) * 128, :])

                scr = ws.tile([128, D_IN], F32, tag="w_scr", name=f"scr_{a}")
                ssum = ws.tile([128, 1], F32, tag="w_sum", name=f"ssum_{a}")
                nc.scalar.activation(
                    scr[:], w_in[:],
                    mybir.ActivationFunctionType.Abs,
                    accum_out=ssum[:],
                )
                scale = ws.tile([128, 1], F32, tag="w_scale",
                                name=f"scale_{a}")
                nc.vector.tensor_scalar(
                    out=scale[:], in0=ssum[:], scalar1=1.0 / D_IN,
                    scalar2=1e-5, op0=mybir.AluOpType.mult,
                    op1=mybir.AluOpType.max,
                )
                nc.vector.tensor_copy(so_col[:, a : a + 1], scale[:])
                hpos = ws.tile([128, 1], F32, tag="w_hpos", name=f"hp_{a}")
                hneg = ws.tile([128, 1], F32, tag="w_hneg", name=f"hn_{a}")
                nc.vector.tensor_scalar_mul(hpos[:], scale[:], 0.5)
                nc.vector.tensor_scalar_mul(hneg[:], scale[:], -0.5)

                # q = (w > 0.5*scale) - (w < -0.5*scale) in bf16 (exact)
                # (strict > matches round-half-even of round(w/s) at 0.5)
                qp = ws.tile([128, D_IN], BF16, tag="w_qp", name=f"qp_{a}")
                nc.vector.tensor_scalar(
                    out=qp[:], in0=w_in[:], scalar1=hpos[:], scalar2=None,
                    op0=mybir.AluOpType.is_gt,
                )
                qn = ws.tile([128, D_IN], BF16, tag="w_qn", name=f"qn_{a}")
                nc.vector.tensor_scalar(
                    out=qn[:], in0=w_in[:], scalar1=hneg[:], scalar2=None,
                    op0=mybir.AluOpType.is_lt,
                )
                q = ws.tile([128, D_IN], BF16, tag="w_q", name=f"q_{a}")
                nc.vector.tensor_sub(q[:], qp[:], qn[:])

                n_idx, o_off = divmod(a * 128, N_SLICE)
                for g in range(2):
                    wt_ps = tp.tile([128, 8, 128], BF16, tag="xtps", bufs=4,
                                    name=f"wpt_{a}_{g}")
                    for j in range(8):
                        k = g * 8 + j
                        nc.tensor.transpose(
                            wt_ps[:, j, :], q[:, k * 128 : (k + 1) * 128],
                            ident_bf[:],
                        )
                    # duplicate q^T into both DoubleRow pair slots
                    nc.scalar.copy(
                        w_dr[n_idx][:, g * 8 : (g + 1) * 8, 0,
                                    o_off : o_off + 128],
                        wt_ps[:],
                    )
                    nc.vector.tensor_copy(
                        w_dr[n_idx][:, g * 8 : (g + 1) * 8, 1,
                                    o_off : o_off + 128],
                        wt_ps[:],
                    )

            def so_slice(n):
                """Broadcast scales of slice n across partitions."""
                soT_sb = ws.tile([4, 128], F32R, tag="soT", name=f"soT_{n}")
                t_ps = ac.tile([128, N_SLICE], F32, tag="acc", bufs=4,
                               name=f"sot_ps_{n}")
                nc.tensor.transpose(
                    t_ps[0:4, 0:128].bitcast(F32R),
                    so_col[:, 4 * n : 4 * n + 4],
                    ident_fr[:],
                )
                nc.scalar.copy(soT_sb[:], t_ps[0:4, 0:128])
                bc = ac.tile([128, N_SLICE], F32, tag="acc", bufs=4,
                             name=f"so_bc_{n}")
                for t in range(4):
                    nc.tensor.matmul(
                        bc[:, t * 128 : (t + 1) * 128],
                        sel[:, t * 128 : (t + 1) * 128],
                        soT_sb[:],
                        start=True, stop=True,
                    )
                nc.scalar.copy(so_full[n][:], bc[:])

            def x_stage(m):
                """Load x row-tile m, bf16, transpose, hi/lo split to fp8."""
                x_in = xs.tile([128, D_IN], F32, tag="x_in", bufs=3,
                               name=f"x_in_{m}")
                nc.sync.dma_start(x_in[:], x_d[m * 128 : (m + 1) * 128, :])
                x_bf = xs.tile([128, D_IN], BF16, tag="x_bf", bufs=2,
                               name=f"x_bf_{m}")
                nc.scalar.copy(x_bf[:], x_in[:])

                x_t = xs.tile([128, K_SUB, 2, 128], F8, tag="x_t", bufs=12,
                              name=f"x_t_{m}")
                for g in range(2):
                    pt = tp.tile([128, 8, 128], BF16, tag="xtps", bufs=4,
                                 name=f"xpt_{m}_{g}")
                    for j in range(8):
                        k = g * 8 + j
                        nc.tensor.transpose(
                            pt[:, j, :], x_bf[:, k * 128 : (k + 1) * 128],
                            ident_bf[:],
                        )
                    hi = x_t[:, g * 8 : (g + 1) * 8, 0, :]
                    lo = x_t[:, g * 8 : (g + 1) * 8, 1, :]
                    nc.vector.tensor_copy(hi, pt[:])
                    nc.vector.tensor_tensor(
                        out=lo, in0=pt[:], in1=hi,
                        op=mybir.AluOpType.subtract,
                    )
                return x_t

            def mm_group(m, n, x_t):
                """One DoubleRow accumulation group + scaled fp16 store."""
                acc = ac.tile([128, N_SLICE], F32, tag="acc", bufs=4,
                              name=f"acc{n}_{m}")
                for k in range(K_SUB):
                    nc.tensor.matmul(
                        acc[:],
                        x_t[:, k, :, :],
                        w_dr[n][:, k, :, :],
                        start=(k == 0),
                        stop=(k == K_SUB - 1),
                        perf_mode=DR,
                    )
                y_sb = ysp.tile([128, N_SLICE], F16, tag="y_sb", bufs=4,
                                name=f"y_sb{n}_{m}")
                nc.vector.tensor_tensor(
                    out=y_sb[:], in0=acc[:], in1=so_full[n][:],
                    op=mybir.AluOpType.mult,
                )
                nc.sync.dma_start(
                    y_d[m * 128 : (m + 1) * 128,
                        n * N_SLICE : (n + 1) * N_SLICE],
                    y_sb[:],
                )

            # ---- schedule ----
            xts = {}
            xts[0] = x_stage(0)
            for a in range(4):
                w_chunk(a)
            so_slice(0)
            xts[1] = x_stage(1)
            xts[2] = x_stage(2)

            # Phase A: n=0 only for m=0..NPRE-1 while W chunks 4..15 land
            wq = list(range(4, O_TILES))
            for m in range(NPRE):
                mm_group(m, 0, xts[m])
                for _ in range(2 if m < 4 else 1):
                    if wq:
                        a = wq.pop(0)
                        w_chunk(a)
                        if a % 4 == 3:
                            so_slice(a // 4)
                if m + 3 < M_TILES:
                    xts[m + 3] = x_stage(m + 3)

            # Phase B: full groups for m>=NPRE, one backlog group per step
            backlog = [(mp, n) for mp in range(NPRE) for n in range(1, 4)]
            for m in range(NPRE, M_TILES):
                for n in range(N_SLICES):
                    mm_group(m, n, xts[m])
                if backlog:
                    mp, n = backlog.pop(0)
                    mm_group(mp, n, xts[mp])
                    if not any(b[0] == mp for b in backlog):
                        del xts[mp]
                if m + 3 < M_TILES:
                    xts[m + 3] = x_stage(m + 3)
            while backlog:
                mp, n = backlog.pop(0)
                mm_group(mp, n, xts[mp])

    nc.compile()
    return nc


_NC_CACHE = None


def _get_nc():
    global _NC_CACHE
    if _NC_CACHE is None:
        _NC_CACHE = _build()
    return _NC_CACHE


def kernel(x: np.ndarray, weight: np.ndarray, _trace: bool = False):
    assert x.shape == (B, S, D_IN) and weight.shape == (D_OUT, D_IN)
    x_flat = np.ascontiguousarray(x.reshape(R, D_IN), dtype=np.float32)
    in_maps = []
    for c in range(NCORES):
        r, col = divmod(c, CGRP)
        in_maps.append(
            {
                "x": np.ascontiguousarray(x_flat[r * R_SH : (r + 1) * R_SH]),
                "w": np.ascontiguousarray(
                    weight[col * O_SH : (col + 1) * O_SH], dtype=np.float32
                ),
            }
        )
    nc = _get_nc()
    res = run_bass_kernel_spmd(
        nc, in_maps, core_ids=list(range(NCORES)), trace=_trace
    )
    y = np.empty((R, D_OUT), dtype=np.float32)
    for c in range(NCORES):
        r, col = divmod(c, CGRP)
        y[r * R_SH : (r + 1) * R_SH, col * O_SH : (col + 1) * O_SH] = (
            res.results[c]["y"]
        )
    out = y.reshape(B, S, D_OUT)
    if _trace:
        return out, res
    return out


# revision 7
# speedup vs baseline: 1.1102x; 1.0837x over previous
"""BitLinear (ternary weight quantization + linear) on 8 TRN2 NeuronCores.

y = x @ w_eff.T with w_eff = clip(round(w/scale), -1, 1) * scale,
scale = clamp(mean |w| per row, 1e-5).

The quantized weight is ternary, so the matmul is
y[m,o] = scale_o * sum_k q[o,k] * x[m,k] with q in {-1,0,1} -- exactly
representable in fp8e4. The PE's fp8 DoubleRow perf mode packs 2
contraction slots per cell (d = w0*m0 + w1*m1) at the same
columns-per-cycle rate as bf16, i.e. 2x the contraction throughput.

Slot budget: full-precision x needs a hi+lo e4m3 pair per k (no win over
bf16), while single e4m3 x measures ~1.8e-2 absmax error -- too close to
the 2e-2 gate. Compromise: x_hi = e4m3(bf16(x)) for all k, plus an
x_lo = e4m3(bf16(x) - x_hi) correction for half the k (k < 1024).
Measured 1.4e-2 absmax vs fp64, and 12 DoubleRow slots per 2048-k group
vs bf16's 16 -> 1.33x less PE matmul time. Products are exact in fp8
(ternary q), accumulation in fp32 PSUM.

Per accumulation group (one 512-wide out slice, K=2048): 8 hi matmuls
pair (k, k+1024), then 4 lo matmuls pair (k, k+512) over k<1024.

Sharding: 2 row-groups x 4 out-groups. Each core: x rows r*4096.. vs w
rows c*2048.. Per-core DMA 32 MiB x + 16 MiB w + 16 MiB y(fp16) = 64 MiB,
well under PE time. Per-row quantization is shard-local; the per-out-row
scale is applied at eviction via so_full (scales broadcast across
partitions once with a tiny f32r selector-matmul).

Schedule: stage x 0-5 and quantize w chunks 0-7 (n-slices 0,1) up front;
phase A runs m=0..7 at n=0,1 while w chunks 8-15 fill slices 2,3; phase B
runs full m-tiles plus one backlog (m<8, n in 2,3) group per step. The
W-chain (DMA -> ACT abs -> DVE ternarize -> PE transpose) pipelines
against the early matmuls so the PE stays fed.
"""

import numpy as np

import concourse.bass as bass
import concourse.mybir as mybir
import concourse.tile as tile
from concourse import bacc
from concourse.bass_utils import run_bass_kernel_spmd
from concourse.masks import make_identity

F32 = mybir.dt.float32
F32R = mybir.dt.float32r
BF16 = mybir.dt.bfloat16
F16 = mybir.dt.float16
F8 = mybir.dt.float8e4
DR = mybir.MatmulPerfMode.DoubleRow

# Problem shape (hardcoded per contract)
B, S, D_IN, D_OUT = 4, 2048, 2048, 8192
NCORES = 8
RGRP, CGRP = 2, 4          # core grid: row-groups x out-groups
R = B * S                  # 8192 rows of x
R_SH = R // RGRP           # 4096 rows per core
O_SH = D_OUT // CGRP       # 2048 out features per core
K_SUB = D_IN // 128        # 16 contraction sub-tiles
M_TILES = R_SH // 128      # 32 row tiles
O_TILES = O_SH // 128      # 16 weight row-tiles per core
N_SLICE = 512              # psum bank width (fp32)
N_SLICES = O_SH // N_SLICE # 4
N_HI = 8                   # hi DoubleRow matmuls per group (all 2048 k)
N_LO = 4                   # lo matmuls per group (k < 1024 corrected)
NPRE = 8                   # m-tiles that run n=0,1 only while W fills


def _build():
    nc = bacc.Bacc(None, target_bir_lowering=False)

    x_d = nc.dram_tensor("x", [R_SH, D_IN], F32, kind="ExternalInput")
    w_d = nc.dram_tensor("w", [O_SH, D_IN], F32, kind="ExternalInput")
    y_d = nc.dram_tensor("y", [R_SH, O_SH], F16, kind="ExternalOutput")

    with tile.TileContext(nc) as tc:
        with (
            tc.tile_pool(name="const", bufs=1) as const,
            tc.tile_pool(name="wt", bufs=1) as wtp,
            tc.tile_pool(name="ws", bufs=1) as ws,
            tc.tile_pool(name="xs", bufs=1) as xs,
            tc.tile_pool(name="ys", bufs=1) as ysp,
            tc.tile_pool(name="tp", bufs=1, space="PSUM") as tp,
            tc.tile_pool(name="ac", bufs=1, space="PSUM") as ac,
        ):
            ident_f = const.tile([128, 128], F32)
            make_identity(nc, ident_f[:])
            ident_bf = const.tile([128, 128], BF16)
            nc.vector.tensor_copy(ident_bf[:], ident_f[:])
            ident_fr = const.tile([128, 128], F32R)
            nc.vector.tensor_copy(ident_fr[:], ident_f[:])
            # sel[k, t*128+p] = (k==t): row-selector for the so broadcast
            sel_f = const.tile([4, 512], F32)
            nc.gpsimd.memset(sel_f[:], 0.0)
            nc.gpsimd.affine_select(
                out=sel_f[:].rearrange("p (t j) -> p t j", t=4),
                in_=sel_f[:].rearrange("p (t j) -> p t j", t=4),
                compare_op=mybir.AluOpType.not_equal,
                fill=1.0,
                base=0,
                pattern=[[-1, 4], [0, 128]],
                channel_multiplier=1,
            )
            sel = const.tile([4, 512], F32R)
            nc.vector.tensor_copy(sel[:], sel_f[:])

            # DoubleRow weight layouts, resident in SBUF (fp8):
            # w_hi[n][ki, i, s, o'] = q^T[s*1024 + i*128 + ki, n*512+o']
            # w_lo[n][ki, j, s, o'] = q^T[s*512 + j*128 + ki, n*512+o']
            w_hi = [
                wtp.tile([128, N_HI, 2, N_SLICE], F8, name=f"whi{n}")
                for n in range(N_SLICES)
            ]
            w_lo = [
                wtp.tile([128, N_LO, 2, N_SLICE], F8, name=f"wlo{n}")
                for n in range(N_SLICES)
            ]
            # so_full[n][p, o'] = scale of out column n*512+o' (any p)
            so_full = [
                wtp.tile([128, N_SLICE], F32, name=f"so{n}")
                for n in range(N_SLICES)
            ]
            so_col = wtp.tile([128, O_TILES], F32R, name="so_col")

            def w_quant(a):
                """DMA + quantize weight rows a*128..(a+1)*128 to ternary."""
                w_in = ws.tile([128, D_IN], F32, tag="w_in", bufs=2,
                               name=f"w_in_{a}")
                nc.sync.dma_start(w_in[:], w_d[a * 128 : (a + 1) * 128, :])

                scr = ws.tile([128, D_IN], F32, tag="w_scr", name=f"scr_{a}")
                ssum = ws.tile([128, 1], F32, tag="w_sum", name=f"ssum_{a}")
                nc.scalar.activation(
                    scr[:], w_in[:],
                    mybir.ActivationFunctionType.Abs,
                    accum_out=ssum[:],
                )
                scale = ws.tile([128, 1], F32, tag="w_scale",
                                name=f"scale_{a}")
                nc.vector.tensor_scalar(
                    out=scale[:], in0=ssum[:], scalar1=1.0 / D_IN,
                    scalar2=1e-5, op0=mybir.AluOpType.mult,
                    op1=mybir.AluOpType.max,
                )
                nc.vector.tensor_copy(so_col[:, a : a + 1], scale[:])
                hpos = ws.tile([128, 1], F32, tag="w_hpos", name=f"hp_{a}")
                hneg = ws.tile([128, 1], F32, tag="w_hneg", name=f"hn_{a}")
                nc.vector.tensor_scalar_mul(hpos[:], scale[:], 0.5)
                nc.vector.tensor_scalar_mul(hneg[:], scale[:], -0.5)

                # q = (w > 0.5*scale) - (w < -0.5*scale) in bf16 (exact)
                # (strict > matches round-half-even of round(w/s) at 0.5)
                qp = ws.tile([128, D_IN], BF16, tag="w_qp", name=f"qp_{a}")
                nc.vector.tensor_scalar(
                    out=qp[:], in0=w_in[:], scalar1=hpos[:], scalar2=None,
                    op0=mybir.AluOpType.is_gt,
                )
                qn = ws.tile([128, D_IN], BF16, tag="w_qn", name=f"qn_{a}")
                nc.vector.tensor_scalar(
                    out=qn[:], in0=w_in[:], scalar1=hneg[:], scalar2=None,
                    op0=mybir.AluOpType.is_lt,
                )
                q = ws.tile([128, D_IN], BF16, tag="w_q", bufs=3,
                            name=f"q_{a}")
                nc.vector.tensor_sub(q[:], qp[:], qn[:])
                return q

            def w_emit(a, q):
                """Transpose ternary q and evict into DoubleRow layouts."""
                n_idx, o_off = divmod(a * 128, N_SLICE)
                for g in range(2):
                    wt_ps = tp.tile([128, 8, 128], BF16, tag="xtps", bufs=4,
                                    name=f"wpt_{a}_{g}")
                    for j in range(8):
                        k = g * 8 + j
                        nc.tensor.transpose(
                            wt_ps[:, j, :], q[:, k * 128 : (k + 1) * 128],
                            ident_bf[:],
                        )
                    osl = slice(o_off, o_off + 128)
                    # hi slot: k16 0..7 -> slot 0, k16 8..15 -> slot 1
                    nc.scalar.copy(w_hi[n_idx][:, :, g, osl], wt_ps[:])
                    if g == 0:
                        # lo slots: k16 0..3 -> slot 0, k16 4..7 -> slot 1
                        nc.vector.tensor_copy(
                            w_lo[n_idx][:, :, 0, osl], wt_ps[:, 0:4, :]
                        )
                        nc.vector.tensor_copy(
                            w_lo[n_idx][:, :, 1, osl], wt_ps[:, 4:8, :]
                        )

            def so_slice(n):
                """Broadcast scales of slice n across partitions."""
                soT_sb = ws.tile([4, 128], F32R, tag="soT", name=f"soT_{n}")
                t_ps = ac.tile([128, N_SLICE], F32, tag="acc", bufs=4,
                               name=f"sot_ps_{n}")
                nc.tensor.transpose(
                    t_ps[0:4, 0:128].bitcast(F32R),
                    so_col[:, 4 * n : 4 * n + 4],
                    ident_fr[:],
                )
                nc.scalar.copy(soT_sb[:], t_ps[0:4, 0:128])
                bc = ac.tile([128, N_SLICE], F32, tag="acc", bufs=4,
                             name=f"so_bc_{n}")
                for t in range(4):
                    nc.tensor.matmul(
                        bc[:, t * 128 : (t + 1) * 128],
                        sel[:, t * 128 : (t + 1) * 128],
                        soT_sb[:],
                        start=True, stop=True,
                    )
                nc.scalar.copy(so_full[n][:], bc[:])

            def x_stage(m):
                """Load x row-tile m, bf16, transpose, hi/lo split to fp8."""
                x_in = xs.tile([128, D_IN], F32, tag="x_in", bufs=3,
                               name=f"x_in_{m}")
                nc.sync.dma_start(x_in[:], x_d[m * 128 : (m + 1) * 128, :])
                x_bf = xs.tile([128, D_IN], BF16, tag="x_bf", bufs=2,
                               name=f"x_bf_{m}")
                nc.scalar.copy(x_bf[:], x_in[:])

                xh = xs.tile([128, N_HI, 2, 128], F8, tag="x_h", bufs=14,
                             name=f"x_h_{m}")
                xl = xs.tile([128, N_LO, 2, 128], F8, tag="x_l", bufs=14,
                             name=f"x_l_{m}")
                for g in range(2):
                    pt = tp.tile([128, 8, 128], BF16, tag="xtps", bufs=4,
                                 name=f"xpt_{m}_{g}")
                    for j in range(8):
                        k = g * 8 + j
                        nc.tensor.transpose(
                            pt[:, j, :], x_bf[:, k * 128 : (k + 1) * 128],
                            ident_bf[:],
                        )
                    hi = xh[:, :, g, :]
                    nc.vector.tensor_copy(hi, pt[:])
                    if g == 0:
                        # lo = bf16(x) - hi for k < 1024, into (k, k+512)
                        nc.vector.tensor_tensor(
                            out=xl[:, :, 0, :], in0=pt[:, 0:4, :],
                            in1=hi[:, 0:4, :],
                            op=mybir.AluOpType.subtract,
                        )
                        nc.vector.tensor_tensor(
                            out=xl[:, :, 1, :], in0=pt[:, 4:8, :],
                            in1=hi[:, 4:8, :],
                            op=mybir.AluOpType.subtract,
                        )
                return xh, xl

            def mm_group(m, n, xt):
                """One 12-matmul DoubleRow group + scaled fp16 store."""
                xh, xl = xt
                acc = ac.tile([128, N_SLICE], F32, tag="acc", bufs=4,
                              name=f"acc{n}_{m}")
                for i in range(N_HI):
                    nc.tensor.matmul(
                        acc[:],
                        xh[:, i, :, :],
                        w_hi[n][:, i, :, :],
                        start=(i == 0),
                        stop=False,
                        perf_mode=DR,
                    )
                for j in range(N_LO):
                    nc.tensor.matmul(
                        acc[:],
                        xl[:, j, :, :],
                        w_lo[n][:, j, :, :],
                        start=False,
                        stop=(j == N_LO - 1),
                        perf_mode=DR,
                    )
                y_sb = ysp.tile([128, N_SLICE], F16, tag="y_sb", bufs=4,
                                name=f"y_sb{n}_{m}")
                nc.vector.tensor_tensor(
                    out=y_sb[:], in0=acc[:], in1=so_full[n][:],
                    op=mybir.AluOpType.mult,
                )
                nc.sync.dma_start(
                    y_d[m * 128 : (m + 1) * 128,
                        n * N_SLICE : (n + 1) * N_SLICE],
                    y_sb[:],
                )

            # ---- schedule ----
            xts = {}
            qs = {}
            # Prologue: x 0-5 staged; w chunks 0-7 (slices 0,1) ready.
            # Quantization (DMA+DVE) runs ~2 chunks ahead of the PE
            # transposes so the PE never waits on the W chain.
            xts[0] = x_stage(0)
            qs[0] = w_quant(0)
            qs[1] = w_quant(1)
            xts[1] = x_stage(1)
            for a in range(8):
                w_emit(a, qs.pop(a))
                if a + 2 < 9:
                    qs[a + 2] = w_quant(a + 2)
                if a + 2 < 6:
                    xts[a + 2] = x_stage(a + 2)
                if a % 4 == 3:
                    so_slice(a // 4)

            # Phase A: n=0,1 for m=0..NPRE-1 while W chunks 8-15 land
            for m in range(NPRE):
                mm_group(m, 0, xts[m])
                a = 8 + m
                w_emit(a, qs.pop(a))
                if a + 1 < O_TILES:
                    qs[a + 1] = w_quant(a + 1)
                if a % 4 == 3:
                    so_slice(a // 4)
                mm_group(m, 1, xts[m])
                if m + 6 < M_TILES:
                    xts[m + 6] = x_stage(m + 6)

            # Phase B: full groups for m>=NPRE, one backlog group per step
            backlog = [(mp, n) for mp in range(NPRE) for n in (2, 3)]
            for m in range(NPRE, M_TILES):
                for n in range(N_SLICES):
                    mm_group(m, n, xts[m])
                if backlog:
                    mp, n = backlog.pop(0)
                    mm_group(mp, n, xts[mp])
                    if not any(b[0] == mp for b in backlog):
                        del xts[mp]
                if m + 6 < M_TILES and (m + 6) not in xts:
                    xts[m + 6] = x_stage(m + 6)
            while backlog:
                mp, n = backlog.pop(0)
                mm_group(mp, n, xts[mp])

    nc.compile()
    return nc


_NC_CACHE = None


def _get_nc():
    global _NC_CACHE
    if _NC_CACHE is None:
        _NC_CACHE = _build()
    return _NC_CACHE


def kernel(x: np.ndarray, weight: np.ndarray, _trace: bool = False):
    assert x.shape == (B, S, D_IN) and weight.shape == (D_OUT, D_IN)
    x_flat = np.ascontiguousarray(x.reshape(R, D_IN), dtype=np.float32)
    in_maps = []
    for c in range(NCORES):
        r, col = divmod(c, CGRP)
        in_maps.append(
            {
                "x": np.ascontiguousarray(x_flat[r * R_SH : (r + 1) * R_SH]),
                "w": np.ascontiguousarray(
                    weight[col * O_SH : (col + 1) * O_SH], dtype=np.float32
                ),
            }
        )
    nc = _get_nc()
    res = run_bass_kernel_spmd(
        nc, in_maps, core_ids=list(range(NCORES)), trace=_trace
    )
    y = np.empty((R, D_OUT), dtype=np.float32)
    for c in range(NCORES):
        r, col = divmod(c, CGRP)
        y[r * R_SH : (r + 1) * R_SH, col * O_SH : (col + 1) * O_SH] = (
            res.results[c]["y"]
        )
    out = y.reshape(B, S, D_OUT)
    if _trace:
        return out, res
    return out


# revision 8
# speedup vs baseline: 1.3162x; 1.1855x over previous
"""BitLinear (ternary weight quantization + linear) on 8 TRN2 NeuronCores.

y = x @ w_eff.T with w_eff = clip(round(w/scale), -1, 1) * scale,
scale = clamp(mean |w| per row, 1e-5).

The quantized weight is ternary, so the matmul is
y[m,o] = scale_o * sum_k q[o,k] * x[m,k] with q in {-1,0,1} -- exactly
representable in fp8e4. The PE's fp8 DoubleRow perf mode packs 2
contraction slots per cell (d = w0*m0 + w1*m1) at the same
columns-per-cycle rate as bf16, i.e. 2x the contraction throughput.

Slot budget: full-precision x needs a hi+lo e4m3 pair per k (no win over
bf16), while single e4m3 x measures ~1.8e-2 absmax error -- too close to
the 2e-2 gate. Compromise: x_hi = e4m3(bf16(x)) for all k, plus an
x_lo = e4m3(bf16(x) - x_hi) correction for half the k (k < 1024).
Measured 1.4e-2 absmax vs fp64, and 12 DoubleRow slots per 2048-k group
vs bf16's 16 -> 1.33x less PE matmul time. Products are exact in fp8
(ternary q), accumulation in fp32 PSUM.

Per accumulation group (one 512-wide out slice, K=2048): 8 hi matmuls
pair (k, k+1024), then 4 lo matmuls pair (k, k+512) over k<1024.

Sharding: 2 row-groups x 4 out-groups. Each core: x rows r*4096.. vs w
rows c*2048.. Per-core DMA 32 MiB x + 16 MiB w + 16 MiB y(fp16) = 64 MiB,
well under PE time. Per-row quantization is shard-local; the per-out-row
scale is applied at eviction via so_full (scales broadcast across
partitions once with a tiny f32r selector-matmul).

Schedule: stage x 0-5 and quantize w chunks 0-7 (n-slices 0,1) up front;
phase A runs m=0..7 at n=0,1 while w chunks 8-15 fill slices 2,3; phase B
runs full m-tiles plus one backlog (m<8, n in 2,3) group per step. The
W-chain (DMA -> ACT abs -> DVE ternarize -> PE transpose) pipelines
against the early matmuls so the PE stays fed.
"""

import numpy as np

import concourse.bass as bass
import concourse.mybir as mybir
import concourse.tile as tile
from concourse import bacc
from concourse.bass_utils import run_bass_kernel_spmd
from concourse.masks import make_identity

F32 = mybir.dt.float32
F32R = mybir.dt.float32r
BF16 = mybir.dt.bfloat16
F16 = mybir.dt.float16
F8 = mybir.dt.float8e4
DR = mybir.MatmulPerfMode.DoubleRow

# Problem shape (hardcoded per contract)
B, S, D_IN, D_OUT = 4, 2048, 2048, 8192
NCORES = 8
RGRP, CGRP = 2, 4          # core grid: row-groups x out-groups
R = B * S                  # 8192 rows of x
R_SH = R // RGRP           # 4096 rows per core
O_SH = D_OUT // CGRP       # 2048 out features per core
K_SUB = D_IN // 128        # 16 contraction sub-tiles
M_TILES = R_SH // 128      # 32 row tiles
O_TILES = O_SH // 128      # 16 weight row-tiles per core
N_SLICE = 512              # psum bank width (fp32)
N_SLICES = O_SH // N_SLICE # 4
N_HI = 8                   # hi DoubleRow matmuls per group (all 2048 k)
N_LO = 4                   # lo matmuls per group (k < 1024 corrected)
NPRE = 8                   # m-tiles that run n=0,1 only while W fills


def _build():
    nc = bacc.Bacc(None, target_bir_lowering=False)

    x_d = nc.dram_tensor("x", [R_SH, D_IN], F32, kind="ExternalInput")
    w_d = nc.dram_tensor("w", [O_SH, D_IN], F32, kind="ExternalInput")
    y_d = nc.dram_tensor("y", [R_SH, O_SH], F16, kind="ExternalOutput")

    with tile.TileContext(nc) as tc:
        with (
            tc.tile_pool(name="const", bufs=1) as const,
            tc.tile_pool(name="wt", bufs=1) as wtp,
            tc.tile_pool(name="ws", bufs=1) as ws,
            tc.tile_pool(name="xs", bufs=1) as xs,
            tc.tile_pool(name="ys", bufs=1) as ysp,
            tc.tile_pool(name="tp", bufs=1, space="PSUM") as tp,
            tc.tile_pool(name="ac", bufs=1, space="PSUM") as ac,
        ):
            ident_f = const.tile([128, 128], F32)
            make_identity(nc, ident_f[:])
            ident_bf = const.tile([128, 128], BF16)
            nc.vector.tensor_copy(ident_bf[:], ident_f[:])
            ident_fr = const.tile([128, 128], F32R)
            nc.vector.tensor_copy(ident_fr[:], ident_f[:])
            # sel[k, t*128+p] = (k==t): row-selector for the so broadcast
            sel_f = const.tile([4, 512], F32)
            nc.gpsimd.memset(sel_f[:], 0.0)
            nc.gpsimd.affine_select(
                out=sel_f[:].rearrange("p (t j) -> p t j", t=4),
                in_=sel_f[:].rearrange("p (t j) -> p t j", t=4),
                compare_op=mybir.AluOpType.not_equal,
                fill=1.0,
                base=0,
                pattern=[[-1, 4], [0, 128]],
                channel_multiplier=1,
            )
            sel = const.tile([4, 512], F32R)
            nc.vector.tensor_copy(sel[:], sel_f[:])

            # DoubleRow weight layout, resident in SBUF (fp8), one tile per
            # n-slice so each 12-matmul group streams consecutive offsets:
            # i in 0..7 (hi):  slot s holds q^T[s*1024 + i*128 + ki]
            # i in 8..11 (lo): slot s holds q^T[s*512 + (i-8)*128 + ki]
            w_all = [
                wtp.tile([128, N_HI + N_LO, 2, N_SLICE], F8, name=f"wal{n}")
                for n in range(N_SLICES)
            ]
            # so_full[n][p, o'] = scale of out column n*512+o' (any p)
            so_full = [
                wtp.tile([128, N_SLICE], F32, name=f"so{n}")
                for n in range(N_SLICES)
            ]
            so_col = wtp.tile([128, O_TILES], F32R, name="so_col")

            def w_quant(a):
                """DMA + quantize weight rows a*128..(a+1)*128 to ternary."""
                w_in = ws.tile([128, D_IN], F32, tag="w_in", bufs=2,
                               name=f"w_in_{a}")
                nc.sync.dma_start(w_in[:], w_d[a * 128 : (a + 1) * 128, :])

                scr = ws.tile([128, D_IN], F32, tag="w_scr", name=f"scr_{a}")
                ssum = ws.tile([128, 1], F32, tag="w_sum", name=f"ssum_{a}")
                nc.scalar.activation(
                    scr[:], w_in[:],
                    mybir.ActivationFunctionType.Abs,
                    accum_out=ssum[:],
                )
                scale = ws.tile([128, 1], F32, tag="w_scale",
                                name=f"scale_{a}")
                nc.vector.tensor_scalar(
                    out=scale[:], in0=ssum[:], scalar1=1.0 / D_IN,
                    scalar2=1e-5, op0=mybir.AluOpType.mult,
                    op1=mybir.AluOpType.max,
                )
                nc.vector.tensor_copy(so_col[:, a : a + 1], scale[:])
                hpos = ws.tile([128, 1], F32, tag="w_hpos", name=f"hp_{a}")
                hneg = ws.tile([128, 1], F32, tag="w_hneg", name=f"hn_{a}")
                nc.vector.tensor_scalar_mul(hpos[:], scale[:], 0.5)
                nc.vector.tensor_scalar_mul(hneg[:], scale[:], -0.5)

                # q = (w > 0.5*scale) - (w < -0.5*scale) in bf16 (exact)
                # (strict > matches round-half-even of round(w/s) at 0.5)
                qp = ws.tile([128, D_IN], BF16, tag="w_qp", name=f"qp_{a}")
                nc.vector.tensor_scalar(
                    out=qp[:], in0=w_in[:], scalar1=hpos[:], scalar2=None,
                    op0=mybir.AluOpType.is_gt,
                )
                qn = ws.tile([128, D_IN], BF16, tag="w_qn", name=f"qn_{a}")
                nc.vector.tensor_scalar(
                    out=qn[:], in0=w_in[:], scalar1=hneg[:], scalar2=None,
                    op0=mybir.AluOpType.is_lt,
                )
                q = ws.tile([128, D_IN], BF16, tag="w_q", bufs=3,
                            name=f"q_{a}")
                nc.vector.tensor_sub(q[:], qp[:], qn[:])
                return q

            def w_emit(a, q):
                """Transpose ternary q and evict into DoubleRow layouts."""
                n_idx, o_off = divmod(a * 128, N_SLICE)
                for g in range(2):
                    wt_ps = tp.tile([128, 8, 128], BF16, tag="xtps", bufs=4,
                                    name=f"wpt_{a}_{g}")
                    for j in range(8):
                        k = g * 8 + j
                        nc.tensor.transpose(
                            wt_ps[:, j, :], q[:, k * 128 : (k + 1) * 128],
                            ident_bf[:],
                        )
                    osl = slice(o_off, o_off + 128)
                    # hi: k16 0..7 -> slot 0, k16 8..15 -> slot 1
                    nc.scalar.copy(
                        w_all[n_idx][:, 0:N_HI, g, osl], wt_ps[:]
                    )
                    if g == 0:
                        # lo: k16 0..3 -> slot 0, k16 4..7 -> slot 1
                        nc.vector.tensor_copy(
                            w_all[n_idx][:, N_HI : N_HI + N_LO, 0, osl],
                            wt_ps[:, 0:4, :],
                        )
                        nc.vector.tensor_copy(
                            w_all[n_idx][:, N_HI : N_HI + N_LO, 1, osl],
                            wt_ps[:, 4:8, :],
                        )

            def so_slice(n):
                """Broadcast scales of slice n across partitions."""
                soT_sb = ws.tile([4, 128], F32R, tag="soT", name=f"soT_{n}")
                t_ps = ac.tile([128, N_SLICE], F32, tag="acc", bufs=4,
                               name=f"sot_ps_{n}")
                nc.tensor.transpose(
                    t_ps[0:4, 0:128].bitcast(F32R),
                    so_col[:, 4 * n : 4 * n + 4],
                    ident_fr[:],
                )
                nc.scalar.copy(soT_sb[:], t_ps[0:4, 0:128])
                bc = ac.tile([128, N_SLICE], F32, tag="acc", bufs=4,
                             name=f"so_bc_{n}")
                for t in range(4):
                    nc.tensor.matmul(
                        bc[:, t * 128 : (t + 1) * 128],
                        sel[:, t * 128 : (t + 1) * 128],
                        soT_sb[:],
                        start=True, stop=True,
                    )
                nc.scalar.copy(so_full[n][:], bc[:])

            def x_stage(m):
                """Load x row-tile m, bf16, transpose, hi/lo split to fp8."""
                x_in = xs.tile([128, D_IN], F32, tag="x_in", bufs=3,
                               name=f"x_in_{m}")
                nc.sync.dma_start(x_in[:], x_d[m * 128 : (m + 1) * 128, :])
                x_bf = xs.tile([128, D_IN], BF16, tag="x_bf", bufs=2,
                               name=f"x_bf_{m}")
                nc.scalar.copy(x_bf[:], x_in[:])

                x_t = xs.tile([128, N_HI + N_LO, 2, 128], F8, tag="x_t",
                              bufs=14, name=f"x_t_{m}")
                for g in range(2):
                    pt = tp.tile([128, 8, 128], BF16, tag="xtps", bufs=4,
                                 name=f"xpt_{m}_{g}")
                    for j in range(8):
                        k = g * 8 + j
                        nc.tensor.transpose(
                            pt[:, j, :], x_bf[:, k * 128 : (k + 1) * 128],
                            ident_bf[:],
                        )
                    hi = x_t[:, 0:N_HI, g, :]
                    nc.vector.tensor_copy(hi, pt[:])
                    if g == 0:
                        # lo = bf16(x) - hi for k < 1024, into (k, k+512)
                        nc.vector.tensor_tensor(
                            out=x_t[:, N_HI : N_HI + N_LO, 0, :],
                            in0=pt[:, 0:4, :], in1=hi[:, 0:4, :],
                            op=mybir.AluOpType.subtract,
                        )
                        nc.vector.tensor_tensor(
                            out=x_t[:, N_HI : N_HI + N_LO, 1, :],
                            in0=pt[:, 4:8, :], in1=hi[:, 4:8, :],
                            op=mybir.AluOpType.subtract,
                        )
                return x_t

            def mm_group(m, n, x_t):
                """One 12-matmul DoubleRow group + scaled fp16 store."""
                nmm = N_HI + N_LO
                acc = ac.tile([128, N_SLICE], F32, tag="acc", bufs=4,
                              name=f"acc{n}_{m}")
                for i in range(nmm):
                    nc.tensor.matmul(
                        acc[:],
                        x_t[:, i, :, :],
                        w_all[n][:, i, :, :],
                        start=(i == 0),
                        stop=(i == nmm - 1),
                        perf_mode=DR,
                    )
                y_sb = ysp.tile([128, N_SLICE], F16, tag="y_sb", bufs=4,
                                name=f"y_sb{n}_{m}")
                nc.vector.tensor_tensor(
                    out=y_sb[:], in0=acc[:], in1=so_full[n][:],
                    op=mybir.AluOpType.mult,
                )
                nc.sync.dma_start(
                    y_d[m * 128 : (m + 1) * 128,
                        n * N_SLICE : (n + 1) * N_SLICE],
                    y_sb[:],
                )

            # ---- schedule ----
            xts = {}
            qs = {}
            # Prologue: x 0-5 staged; w chunks 0-7 (slices 0,1) ready.
            # Quantization (DMA+DVE) runs ~2 chunks ahead of the PE
            # transposes so the PE never waits on the W chain.
            xts[0] = x_stage(0)
            qs[0] = w_quant(0)
            qs[1] = w_quant(1)
            xts[1] = x_stage(1)
            for a in range(8):
                w_emit(a, qs.pop(a))
                if a + 2 < 9:
                    qs[a + 2] = w_quant(a + 2)
                if a + 2 < 6:
                    xts[a + 2] = x_stage(a + 2)
                if a % 4 == 3:
                    so_slice(a // 4)

            # Phase A: n=0,1 for m=0..NPRE-1 while W chunks 8-15 land
            for m in range(NPRE):
                mm_group(m, 0, xts[m])
                a = 8 + m
                w_emit(a, qs.pop(a))
                if a + 1 < O_TILES:
                    qs[a + 1] = w_quant(a + 1)
                if a % 4 == 3:
                    so_slice(a // 4)
                mm_group(m, 1, xts[m])
                if m + 6 < M_TILES:
                    xts[m + 6] = x_stage(m + 6)

            # Phase B: full groups for m>=NPRE, one backlog group per step
            backlog = [(mp, n) for mp in range(NPRE) for n in (2, 3)]
            for m in range(NPRE, M_TILES):
                for n in range(N_SLICES):
                    mm_group(m, n, xts[m])
                if backlog:
                    mp, n = backlog.pop(0)
                    mm_group(mp, n, xts[mp])
                    if not any(b[0] == mp for b in backlog):
                        del xts[mp]
                if m + 6 < M_TILES and (m + 6) not in xts:
                    xts[m + 6] = x_stage(m + 6)
            while backlog:
                mp, n = backlog.pop(0)
                mm_group(mp, n, xts[mp])

    nc.compile()
    return nc


_NC_CACHE = None


def _get_nc():
    global _NC_CACHE
    if _NC_CACHE is None:
        _NC_CACHE = _build()
    return _NC_CACHE


def kernel(x: np.ndarray, weight: np.ndarray, _trace: bool = False):
    assert x.shape == (B, S, D_IN) and weight.shape == (D_OUT, D_IN)
    x_flat = np.ascontiguousarray(x.reshape(R, D_IN), dtype=np.float32)
    in_maps = []
    for c in range(NCORES):
        r, col = divmod(c, CGRP)
        in_maps.append(
            {
                "x": np.ascontiguousarray(x_flat[r * R_SH : (r + 1) * R_SH]),
                "w": np.ascontiguousarray(
                    weight[col * O_SH : (col + 1) * O_SH], dtype=np.float32
                ),
            }
        )
    nc = _get_nc()
    res = run_bass_kernel_spmd(
        nc, in_maps, core_ids=list(range(NCORES)), trace=_trace
    )
    y = np.empty((R, D_OUT), dtype=np.float32)
    for c in range(NCORES):
        r, col = divmod(c, CGRP)
        y[r * R_SH : (r + 1) * R_SH, col * O_SH : (col + 1) * O_SH] = (
            res.results[c]["y"]
        )
    out = y.reshape(B, S, D_OUT)
    if _trace:
        return out, res
    return out


# revision 10
# speedup vs baseline: 1.3840x; 1.0515x over previous
"""BitLinear (ternary weight quantization + linear) on 8 TRN2 NeuronCores.

y = x @ w_eff.T with w_eff = clip(round(w/scale), -1, 1) * scale,
scale = clamp(mean |w| per row, 1e-5).

The quantized weight is ternary, so the matmul is
y[m,o] = scale_o * sum_k q[o,k] * x[m,k] with q in {-1,0,1} -- exactly
representable in fp8e4. The PE's fp8 DoubleRow perf mode packs 2
contraction slots per cell (d = w0*m0 + w1*m1) at the same
columns-per-cycle rate as bf16, i.e. 2x the contraction throughput.

Slot budget: full-precision x needs a hi+lo e4m3 pair per k (no win over
bf16), while single e4m3 x measures ~1.8e-2 absmax error -- too close to
the 2e-2 gate. Compromise: x_hi = e4m3(bf16(x)) for all k, plus an
x_lo = e4m3(bf16(x) - x_hi) correction for half the k (k < 1024).
Measured 1.4e-2 absmax vs fp64, and 12 DoubleRow slots per 2048-k group
vs bf16's 16 -> 1.33x less PE matmul time. Products are exact in fp8
(ternary q), accumulation in fp32 PSUM.

Per accumulation group (one 512-wide out slice, K=2048): 8 hi matmuls
pair (k, k+1024), then 4 lo matmuls pair (k, k+512) over k<1024.

Sharding: 2 row-groups x 4 out-groups. Each core: x rows r*4096.. vs w
rows c*2048.. Per-core DMA 32 MiB x + 16 MiB w + 16 MiB y(fp16) = 64 MiB,
well under PE time. Per-row quantization is shard-local; the per-out-row
scale is applied at eviction via so_full (scales broadcast across
partitions once with a tiny f32r selector-matmul).

Schedule: stage x 0-5 and quantize w chunks 0-7 (n-slices 0,1) up front;
phase A runs m=0..7 at n=0,1 while w chunks 8-15 fill slices 2,3; phase B
runs full m-tiles plus one backlog (m<8, n in 2,3) group per step. The
W-chain (DMA -> ACT abs -> DVE ternarize -> PE transpose) pipelines
against the early matmuls so the PE stays fed.
"""

import numpy as np

import concourse.bass as bass
import concourse.mybir as mybir
import concourse.tile as tile
from concourse import bacc
from concourse.bass_utils import run_bass_kernel_spmd
from concourse.masks import make_identity

F32 = mybir.dt.float32
F32R = mybir.dt.float32r
BF16 = mybir.dt.bfloat16
F16 = mybir.dt.float16
F8 = mybir.dt.float8e4
DR = mybir.MatmulPerfMode.DoubleRow

# Problem shape (hardcoded per contract)
B, S, D_IN, D_OUT = 4, 2048, 2048, 8192
NCORES = 8
RGRP, CGRP = 2, 4          # core grid: row-groups x out-groups
R = B * S                  # 8192 rows of x
R_SH = R // RGRP           # 4096 rows per core
O_SH = D_OUT // CGRP       # 2048 out features per core
K_SUB = D_IN // 128        # 16 contraction sub-tiles
M_TILES = R_SH // 128      # 32 row tiles
O_TILES = O_SH // 128      # 16 weight row-tiles per core
N_SLICE = 512              # psum bank width (fp32)
N_SLICES = O_SH // N_SLICE # 4
N_HI = 8                   # hi DoubleRow matmuls per group (all 2048 k)
N_LO = 4                   # lo matmuls per group (k < 1024 corrected)
NPRE = 8                   # m-tiles that run n=0,1 only while W fills


def _build():
    nc = bacc.Bacc(None, target_bir_lowering=False)

    x_d = nc.dram_tensor("x", [R_SH, D_IN], F32, kind="ExternalInput")
    w_d = nc.dram_tensor("w", [O_SH, D_IN], F32, kind="ExternalInput")
    y_d = nc.dram_tensor("y", [R_SH, O_SH], F16, kind="ExternalOutput")

    with tile.TileContext(nc) as tc:
        with (
            tc.tile_pool(name="const", bufs=1) as const,
            tc.tile_pool(name="wt", bufs=1) as wtp,
            tc.tile_pool(name="ws", bufs=1) as ws,
            tc.tile_pool(name="xs", bufs=1) as xs,
            tc.tile_pool(name="ys", bufs=1) as ysp,
            tc.tile_pool(name="tp", bufs=1, space="PSUM") as tp,
            tc.tile_pool(name="ac", bufs=1, space="PSUM") as ac,
        ):
            ident_f = const.tile([128, 128], F32)
            make_identity(nc, ident_f[:])
            ident_bf = const.tile([128, 128], BF16)
            nc.vector.tensor_copy(ident_bf[:], ident_f[:])
            ident_fr = const.tile([128, 128], F32R)
            nc.vector.tensor_copy(ident_fr[:], ident_f[:])
            # sel[k, t*128+p] = (k==t): row-selector for the so broadcast
            sel_f = const.tile([4, 512], F32)
            nc.gpsimd.memset(sel_f[:], 0.0)
            nc.gpsimd.affine_select(
                out=sel_f[:].rearrange("p (t j) -> p t j", t=4),
                in_=sel_f[:].rearrange("p (t j) -> p t j", t=4),
                compare_op=mybir.AluOpType.not_equal,
                fill=1.0,
                base=0,
                pattern=[[-1, 4], [0, 128]],
                channel_multiplier=1,
            )
            sel = const.tile([4, 512], F32R)
            nc.vector.tensor_copy(sel[:], sel_f[:])

            # DoubleRow weight layout, resident in SBUF (fp8), one tile per
            # n-slice so each 12-matmul group streams consecutive offsets:
            # i in 0..7 (hi):  slot s holds q^T[s*1024 + i*128 + ki]
            # i in 8..11 (lo): slot s holds q^T[s*512 + (i-8)*128 + ki]
            w_all = [
                wtp.tile([128, N_HI + N_LO, 2, N_SLICE], F8, name=f"wal{n}")
                for n in range(N_SLICES)
            ]
            # so_full[n][p, o'] = scale of out column n*512+o' (any p)
            so_full = [
                wtp.tile([128, N_SLICE], F32, name=f"so{n}")
                for n in range(N_SLICES)
            ]
            so_col = wtp.tile([128, O_TILES], F32R, name="so_col")

            def w_quant(a):
                """DMA + quantize weight rows a*128..(a+1)*128 to ternary."""
                w_in = ws.tile([128, D_IN], F32, tag="w_in", bufs=2,
                               name=f"w_in_{a}")
                nc.sync.dma_start(w_in[:], w_d[a * 128 : (a + 1) * 128, :])

                scr = ws.tile([128, D_IN], F32, tag="w_scr", name=f"scr_{a}")
                ssum = ws.tile([128, 1], F32, tag="w_sum", name=f"ssum_{a}")
                nc.scalar.activation(
                    scr[:], w_in[:],
                    mybir.ActivationFunctionType.Abs,
                    accum_out=ssum[:],
                )
                scale = ws.tile([128, 1], F32, tag="w_scale",
                                name=f"scale_{a}")
                nc.vector.tensor_scalar(
                    out=scale[:], in0=ssum[:], scalar1=1.0 / D_IN,
                    scalar2=1e-5, op0=mybir.AluOpType.mult,
                    op1=mybir.AluOpType.max,
                )
                nc.vector.tensor_copy(so_col[:, a : a + 1], scale[:])
                hpos = ws.tile([128, 1], F32, tag="w_hpos", name=f"hp_{a}")
                hneg = ws.tile([128, 1], F32, tag="w_hneg", name=f"hn_{a}")
                nc.vector.tensor_scalar_mul(hpos[:], scale[:], 0.5)
                nc.vector.tensor_scalar_mul(hneg[:], scale[:], -0.5)

                # q = (w > 0.5*scale) - (w < -0.5*scale) in bf16 (exact)
                # (strict > matches round-half-even of round(w/s) at 0.5)
                qp = ws.tile([128, D_IN], BF16, tag="w_qp", name=f"qp_{a}")
                nc.vector.tensor_scalar(
                    out=qp[:], in0=w_in[:], scalar1=hpos[:], scalar2=None,
                    op0=mybir.AluOpType.is_gt,
                )
                qn = ws.tile([128, D_IN], BF16, tag="w_qn", name=f"qn_{a}")
                nc.vector.tensor_scalar(
                    out=qn[:], in0=w_in[:], scalar1=hneg[:], scalar2=None,
                    op0=mybir.AluOpType.is_lt,
                )
                q = ws.tile([128, D_IN], BF16, tag="w_q", bufs=3,
                            name=f"q_{a}")
                nc.vector.tensor_sub(q[:], qp[:], qn[:])
                return q

            def w_emit(a, q):
                """Transpose ternary q and evict into DoubleRow layouts."""
                n_idx, o_off = divmod(a * 128, N_SLICE)
                for g in range(2):
                    wt_ps = tp.tile([128, 8, 128], BF16, tag="xtps", bufs=4,
                                    name=f"wpt_{a}_{g}")
                    for j in range(8):
                        k = g * 8 + j
                        nc.tensor.transpose(
                            wt_ps[:, j, :], q[:, k * 128 : (k + 1) * 128],
                            ident_bf[:],
                        )
                    osl = slice(o_off, o_off + 128)
                    # hi: k16 0..7 -> slot 0, k16 8..15 -> slot 1
                    nc.scalar.copy(
                        w_all[n_idx][:, 0:N_HI, g, osl], wt_ps[:]
                    )
                    if g == 0:
                        # lo: k16 0..3 -> slot 0, k16 4..7 -> slot 1
                        nc.vector.tensor_copy(
                            w_all[n_idx][:, N_HI : N_HI + N_LO, 0, osl],
                            wt_ps[:, 0:4, :],
                        )
                        nc.vector.tensor_copy(
                            w_all[n_idx][:, N_HI : N_HI + N_LO, 1, osl],
                            wt_ps[:, 4:8, :],
                        )

            def so_slice(n):
                """Broadcast scales of slice n across partitions."""
                soT_sb = ws.tile([4, 128], F32R, tag="soT", name=f"soT_{n}")
                t_ps = ac.tile([128, N_SLICE], F32, tag="acc", bufs=4,
                               name=f"sot_ps_{n}")
                nc.tensor.transpose(
                    t_ps[0:4, 0:128].bitcast(F32R),
                    so_col[:, 4 * n : 4 * n + 4],
                    ident_fr[:],
                )
                nc.scalar.copy(soT_sb[:], t_ps[0:4, 0:128])
                bc = ac.tile([128, N_SLICE], F32, tag="acc", bufs=4,
                             name=f"so_bc_{n}")
                for t in range(4):
                    nc.tensor.matmul(
                        bc[:, t * 128 : (t + 1) * 128],
                        sel[:, t * 128 : (t + 1) * 128],
                        soT_sb[:],
                        start=True, stop=True,
                    )
                nc.scalar.copy(so_full[n][:], bc[:])

            def x_stage(m):
                """Load x row-tile m, bf16, transpose, hi/lo split to fp8."""
                x_in = xs.tile([128, D_IN], F32, tag="x_in", bufs=3,
                               name=f"x_in_{m}")
                nc.sync.dma_start(x_in[:], x_d[m * 128 : (m + 1) * 128, :])
                x_bf = xs.tile([128, D_IN], BF16, tag="x_bf", bufs=2,
                               name=f"x_bf_{m}")
                nc.scalar.copy(x_bf[:], x_in[:])

                x_t = xs.tile([128, N_HI + N_LO, 2, 128], F8, tag="x_t",
                              bufs=14, name=f"x_t_{m}")
                for g in range(2):
                    pt = tp.tile([128, 8, 128], BF16, tag="xtps", bufs=4,
                                 name=f"xpt_{m}_{g}")
                    for j in range(8):
                        k = g * 8 + j
                        nc.tensor.transpose(
                            pt[:, j, :], x_bf[:, k * 128 : (k + 1) * 128],
                            ident_bf[:],
                        )
                    hi = x_t[:, 0:N_HI, g, :]
                    nc.vector.tensor_copy(hi, pt[:])
                    if g == 0:
                        # lo = bf16(x) - hi for k < 1024, into (k, k+512)
                        nc.vector.tensor_tensor(
                            out=x_t[:, N_HI : N_HI + N_LO, 0, :],
                            in0=pt[:, 0:4, :], in1=hi[:, 0:4, :],
                            op=mybir.AluOpType.subtract,
                        )
                        nc.vector.tensor_tensor(
                            out=x_t[:, N_HI : N_HI + N_LO, 1, :],
                            in0=pt[:, 4:8, :], in1=hi[:, 4:8, :],
                            op=mybir.AluOpType.subtract,
                        )
                return x_t

            def mm_group(m, n, x_t):
                """One 12-matmul DoubleRow group + scaled fp16 store."""
                nmm = N_HI + N_LO
                acc = ac.tile([128, N_SLICE], F32, tag="acc", bufs=4,
                              name=f"acc{n}_{m}")
                for i in range(nmm):
                    nc.tensor.matmul(
                        acc[:],
                        x_t[:, i, :, :],
                        w_all[n][:, i, :, :],
                        start=(i == 0),
                        stop=(i == nmm - 1),
                        perf_mode=DR,
                    )
                y_sb = ysp.tile([128, N_SLICE], F16, tag="y_sb", bufs=6,
                                name=f"y_sb{n}_{m}")
                nc.vector.tensor_tensor(
                    out=y_sb[:], in0=acc[:], in1=so_full[n][:],
                    op=mybir.AluOpType.mult,
                )
                nc.sync.dma_start(
                    y_d[m * 128 : (m + 1) * 128,
                        n * N_SLICE : (n + 1) * N_SLICE],
                    y_sb[:],
                )

            # ---- schedule ----
            # Merged pipeline: per step, one W chunk advances (quant 2
            # ahead of its PE transposes), one x tile stages, and up to
            # two matmul groups run as their (x_t, w slice) pair becomes
            # ready. This keeps the DMA (x+w+y) and PE (transposes+mm)
            # both ~80-90% loaded through the fill; phase B then runs
            # PE-bound with the leftover groups drained one per step.
            xts = {}
            qs = {}
            ready_n = set()
            pend = []           # mm groups still to run for m < NPRE
            xts[0] = x_stage(0)
            qs[0] = w_quant(0)
            qs[1] = w_quant(1)
            xts[1] = x_stage(1)

            def run_avail(budget):
                ran = 0
                for mn in list(pend):
                    if ran >= budget:
                        break
                    m, n = mn
                    if n in ready_n and m in xts:
                        mm_group(m, n, xts[m])
                        pend.remove(mn)
                        ran += 1
                return ran

            for s_ in range(O_TILES):
                w_emit(s_, qs.pop(s_))
                if s_ + 2 < O_TILES:
                    qs[s_ + 2] = w_quant(s_ + 2)
                if s_ % 4 == 3:
                    so_slice(s_ // 4)
                    n = s_ // 4
                    ready_n.add(n)
                    pend.extend((m, n) for m in range(NPRE))
                if s_ + 2 < NPRE + 2:
                    xts[s_ + 2] = x_stage(s_ + 2)
                run_avail(2)

            # Phase B: full groups for m>=NPRE plus backlog drain
            for m in range(NPRE, M_TILES):
                if m not in xts:
                    xts[m] = x_stage(m)
                for n in range(N_SLICES):
                    mm_group(m, n, xts[m])
                run_avail(1)
                for mp in list(range(NPRE)):
                    if mp in xts and not any(b[0] == mp for b in pend):
                        del xts[mp]
                if m + 2 < M_TILES and (m + 2) not in xts:
                    xts[m + 2] = x_stage(m + 2)
            while pend:
                run_avail(len(pend))

    nc.compile()
    return nc


_NC_CACHE = None


def _get_nc():
    global _NC_CACHE
    if _NC_CACHE is None:
        _NC_CACHE = _build()
    return _NC_CACHE


def kernel(x: np.ndarray, weight: np.ndarray, _trace: bool = False):
    assert x.shape == (B, S, D_IN) and weight.shape == (D_OUT, D_IN)
    x_flat = np.ascontiguousarray(x.reshape(R, D_IN), dtype=np.float32)
    in_maps = []
    for c in range(NCORES):
        r, col = divmod(c, CGRP)
        in_maps.append(
            {
                "x": np.ascontiguousarray(x_flat[r * R_SH : (r + 1) * R_SH]),
                "w": np.ascontiguousarray(
                    weight[col * O_SH : (col + 1) * O_SH], dtype=np.float32
                ),
            }
        )
    nc = _get_nc()
    res = run_bass_kernel_spmd(
        nc, in_maps, core_ids=list(range(NCORES)), trace=_trace
    )
    y = np.empty((R, D_OUT), dtype=np.float32)
    for c in range(NCORES):
        r, col = divmod(c, CGRP)
        y[r * R_SH : (r + 1) * R_SH, col * O_SH : (col + 1) * O_SH] = (
            res.results[c]["y"]
        )
    out = y.reshape(B, S, D_OUT)
    if _trace:
        return out, res
    return out


# revision 12
# speedup vs baseline: 1.4049x; 1.0151x over previous
"""BitLinear (ternary weight quantization + linear) on 8 TRN2 NeuronCores.

y = x @ w_eff.T with w_eff = clip(round(w/scale), -1, 1) * scale,
scale = clamp(mean |w| per row, 1e-5).

The quantized weight is ternary, so the matmul is
y[m,o] = scale_o * sum_k q[o,k] * x[m,k] with q in {-1,0,1} -- exactly
representable in fp8e4. The PE's fp8 DoubleRow perf mode packs 2
contraction slots per cell (d = w0*m0 + w1*m1) at the same
columns-per-cycle rate as bf16, i.e. 2x the contraction throughput.

Slot budget: full-precision x needs a hi+lo e4m3 pair per k (no win over
bf16), while single e4m3 x measures ~1.8e-2 absmax error -- too close to
the 2e-2 gate. Compromise: x_hi = e4m3(bf16(x)) for all k, plus an
x_lo = e4m3(bf16(x) - x_hi) correction for half the k (k < 1024).
Measured 1.4e-2 absmax vs fp64, and 12 DoubleRow slots per 2048-k group
vs bf16's 16 -> 1.33x less PE matmul time. Products are exact in fp8
(ternary q), accumulation in fp32 PSUM.

Per accumulation group (one 512-wide out slice, K=2048): 8 hi matmuls
pair (k, k+1024), then 4 lo matmuls pair (k, k+512) over k<1024.

Sharding: 2 row-groups x 4 out-groups. Each core: x rows r*4096.. vs w
rows c*2048.. Per-core DMA 32 MiB x + 16 MiB w + 16 MiB y(fp16) = 64 MiB,
well under PE time. Per-row quantization is shard-local; the per-out-row
scale is applied at eviction via so_full (scales broadcast across
partitions once with a tiny f32r selector-matmul).

Schedule: stage x 0-5 and quantize w chunks 0-7 (n-slices 0,1) up front;
phase A runs m=0..7 at n=0,1 while w chunks 8-15 fill slices 2,3; phase B
runs full m-tiles plus one backlog (m<8, n in 2,3) group per step. The
W-chain (DMA -> ACT abs -> DVE ternarize -> PE transpose) pipelines
against the early matmuls so the PE stays fed.
"""

import numpy as np

import concourse.bass as bass
import concourse.mybir as mybir
import concourse.tile as tile
from concourse import bacc
from concourse.bass_utils import run_bass_kernel_spmd
from concourse.masks import make_identity

F32 = mybir.dt.float32
F32R = mybir.dt.float32r
BF16 = mybir.dt.bfloat16
F16 = mybir.dt.float16
F8 = mybir.dt.float8e4
DR = mybir.MatmulPerfMode.DoubleRow

# Problem shape (hardcoded per contract)
B, S, D_IN, D_OUT = 4, 2048, 2048, 8192
NCORES = 8
RGRP, CGRP = 2, 4          # core grid: row-groups x out-groups
R = B * S                  # 8192 rows of x
R_SH = R // RGRP           # 4096 rows per core
O_SH = D_OUT // CGRP       # 2048 out features per core
K_SUB = D_IN // 128        # 16 contraction sub-tiles
M_TILES = R_SH // 128      # 32 row tiles
O_TILES = O_SH // 128      # 16 weight row-tiles per core
N_SLICE = 512              # psum bank width (fp32)
N_SLICES = O_SH // N_SLICE # 4
N_HI = 8                   # hi DoubleRow matmuls per group (all 2048 k)
N_LO = 4                   # lo matmuls per group (k < 1024 corrected)
NPRE = 8                   # m-tiles that run n=0,1 only while W fills


def _build():
    nc = bacc.Bacc(None, target_bir_lowering=False)

    x_d = nc.dram_tensor("x", [R_SH, D_IN], F32, kind="ExternalInput")
    w_d = nc.dram_tensor("w", [O_SH, D_IN], F32, kind="ExternalInput")
    y_d = nc.dram_tensor("y", [R_SH, O_SH], F16, kind="ExternalOutput")

    with tile.TileContext(nc) as tc:
        with (
            tc.tile_pool(name="const", bufs=1) as const,
            tc.tile_pool(name="wt", bufs=1) as wtp,
            tc.tile_pool(name="ws", bufs=1) as ws,
            tc.tile_pool(name="xs", bufs=1) as xs,
            tc.tile_pool(name="ys", bufs=1) as ysp,
            tc.tile_pool(name="tp", bufs=1, space="PSUM") as tp,
            tc.tile_pool(name="ac", bufs=1, space="PSUM") as ac,
        ):
            # issue the first x/w DMAs before any const setup so the
            # loads overlap the gpsimd/DVE preamble
            xin0 = xs.tile([128, D_IN], F32, tag="x_in", bufs=4,
                           name="x_in_0")
            nc.sync.dma_start(xin0[:], x_d[0:128, :])
            xin1 = xs.tile([128, D_IN], F32, tag="x_in", bufs=4,
                           name="x_in_1")
            nc.sync.dma_start(xin1[:], x_d[128:256, :])

            ident_f = const.tile([128, 128], F32)
            make_identity(nc, ident_f[:])
            ident_bf = const.tile([128, 128], BF16)
            nc.vector.tensor_copy(ident_bf[:], ident_f[:])
            ident_fr = const.tile([128, 128], F32R)
            nc.vector.tensor_copy(ident_fr[:], ident_f[:])
            # sel[k, t*128+p] = (k==t): row-selector for the so broadcast
            sel_f = const.tile([4, 512], F32)
            nc.gpsimd.memset(sel_f[:], 0.0)
            nc.gpsimd.affine_select(
                out=sel_f[:].rearrange("p (t j) -> p t j", t=4),
                in_=sel_f[:].rearrange("p (t j) -> p t j", t=4),
                compare_op=mybir.AluOpType.not_equal,
                fill=1.0,
                base=0,
                pattern=[[-1, 4], [0, 128]],
                channel_multiplier=1,
            )
            sel = const.tile([4, 512], F32R)
            nc.vector.tensor_copy(sel[:], sel_f[:])

            # DoubleRow weight layout, resident in SBUF (fp8), one tile per
            # n-slice so each 12-matmul group streams consecutive offsets:
            # i in 0..7 (hi):  slot s holds q^T[s*1024 + i*128 + ki]
            # i in 8..11 (lo): slot s holds q^T[s*512 + (i-8)*128 + ki]
            w_all = [
                wtp.tile([128, N_HI + N_LO, 2, N_SLICE], F8, name=f"wal{n}")
                for n in range(N_SLICES)
            ]
            # so_full[n][p, o'] = scale of out column n*512+o' (any p)
            so_full = [
                wtp.tile([128, N_SLICE], F32, name=f"so{n}")
                for n in range(N_SLICES)
            ]
            so_col = wtp.tile([128, O_TILES], F32R, name="so_col")

            def w_quant(a):
                """DMA + quantize weight rows a*128..(a+1)*128 to ternary."""
                w_in = ws.tile([128, D_IN], F32, tag="w_in", bufs=2,
                               name=f"w_in_{a}")
                nc.sync.dma_start(w_in[:], w_d[a * 128 : (a + 1) * 128, :])

                scr = ws.tile([128, D_IN], F32, tag="w_scr", name=f"scr_{a}")
                ssum = ws.tile([128, 1], F32, tag="w_sum", name=f"ssum_{a}")
                nc.scalar.activation(
                    scr[:], w_in[:],
                    mybir.ActivationFunctionType.Abs,
                    accum_out=ssum[:],
                )
                scale = ws.tile([128, 1], F32, tag="w_scale",
                                name=f"scale_{a}")
                nc.vector.tensor_scalar(
                    out=scale[:], in0=ssum[:], scalar1=1.0 / D_IN,
                    scalar2=1e-5, op0=mybir.AluOpType.mult,
                    op1=mybir.AluOpType.max,
                )
                nc.vector.tensor_copy(so_col[:, a : a + 1], scale[:])
                hpos = ws.tile([128, 1], F32, tag="w_hpos", name=f"hp_{a}")
                hneg = ws.tile([128, 1], F32, tag="w_hneg", name=f"hn_{a}")
                nc.vector.tensor_scalar_mul(hpos[:], scale[:], 0.5)
                nc.vector.tensor_scalar_mul(hneg[:], scale[:], -0.5)

                # q = (w > 0.5*scale) - (w < -0.5*scale) in bf16 (exact)
                # (strict > matches round-half-even of round(w/s) at 0.5)
                qp = ws.tile([128, D_IN], BF16, tag="w_qp", name=f"qp_{a}")
                nc.vector.tensor_scalar(
                    out=qp[:], in0=w_in[:], scalar1=hpos[:], scalar2=None,
                    op0=mybir.AluOpType.is_gt,
                )
                qn = ws.tile([128, D_IN], BF16, tag="w_qn", name=f"qn_{a}")
                nc.vector.tensor_scalar(
                    out=qn[:], in0=w_in[:], scalar1=hneg[:], scalar2=None,
                    op0=mybir.AluOpType.is_lt,
                )
                q = ws.tile([128, D_IN], BF16, tag="w_q", bufs=3,
                            name=f"q_{a}")
                nc.vector.tensor_sub(q[:], qp[:], qn[:])
                return q

            def w_emit(a, q):
                """Transpose ternary q and evict into DoubleRow layouts."""
                n_idx, o_off = divmod(a * 128, N_SLICE)
                for g in range(2):
                    wt_ps = tp.tile([128, 8, 128], BF16, tag="xtps", bufs=4,
                                    name=f"wpt_{a}_{g}")
                    for j in range(8):
                        k = g * 8 + j
                        nc.tensor.transpose(
                            wt_ps[:, j, :], q[:, k * 128 : (k + 1) * 128],
                            ident_bf[:],
                        )
                    osl = slice(o_off, o_off + 128)
                    # hi: k16 0..7 -> slot 0, k16 8..15 -> slot 1
                    nc.scalar.copy(
                        w_all[n_idx][:, 0:N_HI, g, osl], wt_ps[:]
                    )
                    if g == 0:
                        # lo: k16 0..3 -> slot 0, k16 4..7 -> slot 1
                        nc.vector.tensor_copy(
                            w_all[n_idx][:, N_HI : N_HI + N_LO, 0, osl],
                            wt_ps[:, 0:4, :],
                        )
                        nc.vector.tensor_copy(
                            w_all[n_idx][:, N_HI : N_HI + N_LO, 1, osl],
                            wt_ps[:, 4:8, :],
                        )

            def so_slice(n):
                """Broadcast scales of slice n across partitions."""
                soT_sb = ws.tile([4, 128], F32R, tag="soT", name=f"soT_{n}")
                t_ps = ac.tile([128, N_SLICE], F32, tag="acc", bufs=4,
                               name=f"sot_ps_{n}")
                nc.tensor.transpose(
                    t_ps[0:4, 0:128].bitcast(F32R),
                    so_col[:, 4 * n : 4 * n + 4],
                    ident_fr[:],
                )
                nc.scalar.copy(soT_sb[:], t_ps[0:4, 0:128])
                bc = ac.tile([128, N_SLICE], F32, tag="acc", bufs=4,
                             name=f"so_bc_{n}")
                for t in range(4):
                    nc.tensor.matmul(
                        bc[:, t * 128 : (t + 1) * 128],
                        sel[:, t * 128 : (t + 1) * 128],
                        soT_sb[:],
                        start=True, stop=True,
                    )
                nc.scalar.copy(so_full[n][:], bc[:])

            def x_prefetch(m):
                x_in = xs.tile([128, D_IN], F32, tag="x_in", bufs=4,
                               name=f"x_in_{m}")
                nc.sync.dma_start(x_in[:], x_d[m * 128 : (m + 1) * 128, :])
                return x_in

            def x_stage(m, x_in=None):
                """Load x row-tile m, bf16, transpose, hi/lo split to fp8."""
                if x_in is None:
                    x_in = x_prefetch(m)
                x_bf = xs.tile([128, D_IN], BF16, tag="x_bf", bufs=2,
                               name=f"x_bf_{m}")

                x_t = xs.tile([128, N_HI + N_LO, 2, 128], F8, tag="x_t",
                              bufs=14, name=f"x_t_{m}")
                for g in range(2):
                    gsl = slice(g * 1024, (g + 1) * 1024)
                    nc.scalar.copy(x_bf[:, gsl], x_in[:, gsl])
                    pt = tp.tile([128, 8, 128], BF16, tag="xtps", bufs=4,
                                 name=f"xpt_{m}_{g}")
                    for j in range(8):
                        k = g * 8 + j
                        nc.tensor.transpose(
                            pt[:, j, :], x_bf[:, k * 128 : (k + 1) * 128],
                            ident_bf[:],
                        )
                    hi = x_t[:, 0:N_HI, g, :]
                    nc.vector.tensor_copy(hi, pt[:])
                    if g == 0:
                        # lo = bf16(x) - hi for k < 1024, into (k, k+512)
                        nc.vector.tensor_tensor(
                            out=x_t[:, N_HI : N_HI + N_LO, 0, :],
                            in0=pt[:, 0:4, :], in1=hi[:, 0:4, :],
                            op=mybir.AluOpType.subtract,
                        )
                        nc.vector.tensor_tensor(
                            out=x_t[:, N_HI : N_HI + N_LO, 1, :],
                            in0=pt[:, 4:8, :], in1=hi[:, 4:8, :],
                            op=mybir.AluOpType.subtract,
                        )
                return x_t

            def mm_group(m, n, x_t):
                """One 12-matmul DoubleRow group + scaled fp16 store."""
                nmm = N_HI + N_LO
                acc = ac.tile([128, N_SLICE], F32, tag="acc", bufs=4,
                              name=f"acc{n}_{m}")
                for i in range(nmm):
                    nc.tensor.matmul(
                        acc[:],
                        x_t[:, i, :, :],
                        w_all[n][:, i, :, :],
                        start=(i == 0),
                        stop=(i == nmm - 1),
                        perf_mode=DR,
                    )
                y_sb = ysp.tile([128, N_SLICE], F16, tag="y_sb", bufs=6,
                                name=f"y_sb{n}_{m}")
                nc.vector.tensor_tensor(
                    out=y_sb[:], in0=acc[:], in1=so_full[n][:],
                    op=mybir.AluOpType.mult,
                )
                nc.sync.dma_start(
                    y_d[m * 128 : (m + 1) * 128,
                        n * N_SLICE : (n + 1) * N_SLICE],
                    y_sb[:],
                )

            # ---- schedule ----
            # Merged pipeline: per step, one W chunk advances (quant 2
            # ahead of its PE transposes), one x tile stages, and up to
            # two matmul groups run as their (x_t, w slice) pair becomes
            # ready. This keeps the DMA (x+w+y) and PE (transposes+mm)
            # both ~80-90% loaded through the fill; phase B then runs
            # PE-bound with the leftover groups drained one per step.
            xts = {}
            qs = {}
            ready_n = set()
            pend = []           # mm groups still to run for m < NPRE
            xts[0] = x_stage(0, xin0)
            qs[0] = w_quant(0)
            qs[1] = w_quant(1)
            xts[1] = x_stage(1, xin1)
            xq = list(range(2, NPRE + 2))   # x tiles to stage during fill

            def run_avail(budget):
                ran = 0
                for mn in list(pend):
                    if ran >= budget:
                        break
                    m, n = mn
                    if n in ready_n and m in xts:
                        mm_group(m, n, xts[m])
                        pend.remove(mn)
                        ran += 1
                return ran

            for s_ in range(O_TILES):
                w_emit(s_, qs.pop(s_))
                if s_ + 2 < O_TILES:
                    qs[s_ + 2] = w_quant(s_ + 2)
                if s_ % 4 == 3:
                    so_slice(s_ // 4)
                    n = s_ // 4
                    ready_n.add(n)
                    pend.extend((m, n) for m in range(NPRE))
                for _ in range(2 if s_ < 2 else 1):
                    if xq:
                        mx = xq.pop(0)
                        xts[mx] = x_stage(mx)
                run_avail(2)

            # Phase B: full groups for m>=NPRE plus backlog drain
            for m in range(NPRE, M_TILES):
                if m not in xts:
                    xts[m] = x_stage(m)
                for n in range(N_SLICES):
                    mm_group(m, n, xts[m])
                run_avail(1)
                for mp in list(range(NPRE)):
                    if mp in xts and not any(b[0] == mp for b in pend):
                        del xts[mp]
                if m + 2 < M_TILES and (m + 2) not in xts:
                    xts[m + 2] = x_stage(m + 2)
            while pend:
                run_avail(len(pend))

    nc.compile()
    return nc


_NC_CACHE = None


def _get_nc():
    global _NC_CACHE
    if _NC_CACHE is None:
        _NC_CACHE = _build()
    return _NC_CACHE


def kernel(x: np.ndarray, weight: np.ndarray, _trace: bool = False):
    assert x.shape == (B, S, D_IN) and weight.shape == (D_OUT, D_IN)
    x_flat = np.ascontiguousarray(x.reshape(R, D_IN), dtype=np.float32)
    in_maps = []
    for c in range(NCORES):
        r, col = divmod(c, CGRP)
        in_maps.append(
            {
                "x": np.ascontiguousarray(x_flat[r * R_SH : (r + 1) * R_SH]),
                "w": np.ascontiguousarray(
                    weight[col * O_SH : (col + 1) * O_SH], dtype=np.float32
                ),
            }
        )
    nc = _get_nc()
    res = run_bass_kernel_spmd(
        nc, in_maps, core_ids=list(range(NCORES)), trace=_trace
    )
    y = np.empty((R, D_OUT), dtype=np.float32)
    for c in range(NCORES):
        r, col = divmod(c, CGRP)
        y[r * R_SH : (r + 1) * R_SH, col * O_SH : (col + 1) * O_SH] = (
            res.results[c]["y"]
        )
    out = y.reshape(B, S, D_OUT)
    if _trace:
        return out, res
    return out
